# revision 36
# baseline (speedup 1.0000x reference)
"""NonLocalBlock (spatial self-attention) Trainium2 Bass kernel.

Data-parallel over batch: B=8 -> one batch element per NeuronCore.

Per-core computation (C=512, CR=128, N=4096 = 64*64 tokens), all on one core:
  proj = w_in @ x          -> [384, N]; Q=proj[0:128], K=proj[128:256], V=proj[256:384]
  S^T[m,n] = sum_c K[c,m] Q[c,n]    (tiles: m on partitions, n on free axis)
  P = exp(S^T - 8)         (no max subtraction; logits are ~N(0, 2.3), max ~16;
                            the -8 keeps fp8e5m2 away from overflow-to-inf and
                            cancels exactly in y/s)
  s[n] = sum_m P[m,n]      (fp8 DoubleRow ones-matmuls, PSUM-accumulated)
  yT[c,n] = sum_m V^T[m,c] P[m,n]   (fp8 DoubleRow, PSUM-accumulated)
  out = x + w_out @ (yT / s)

Engine plan: QK runs in bf16 (logit precision), the exp output is fp8e5m2 so
PV and the softmax sums contract two m-chunks per DoubleRow matmul. The whole
kernel is one software-pipelined loop: the input projection and V^T transpose
work items carry deadlines and are drip-fed into the attention loop, so the
PE works through them while ScalarE streams exps; PV trails its exp by
PV_DELAY groups; each n-tile's normalize + output projection trail further.
The host supplies x twice: bf16 in a DMA-friendly tiled layout (feeds the
projection) and fp32 row-major (residual, needed much later).
"""

import os
from contextlib import ExitStack

import numpy as np
import ml_dtypes

import concourse.tile as tile
from concourse import bacc, mybir
from concourse.bass_utils import run_bass_kernel_spmd
from concourse.masks import make_identity

FP32 = mybir.dt.float32
BF16 = mybir.dt.bfloat16
FP8W = mybir.dt.float8e5  # e5m2: exp weights (wide dynamic range)
FP8V = mybir.dt.float8e4  # e4m3: V^T values
EXP_BIAS = -8.0

B, C, HH, WW = 8, 512, 64, 64
N = HH * WW          # 4096 spatial tokens
CR = 128             # reduced channels (= partition count, exact fit)
P = 128              # SBUF partitions
CCH = C // P         # 4 chunks of input channels
NT = 512             # n-tile width (one PSUM bank of fp32)
N_TILES = N // NT    # 8
M_CHUNKS = N // P    # 32 chunks of the m (key/value token) axis
GROUP = 2            # m-chunks per exp batch (one fp8 DoubleRow pair)
N_GROUPS = M_CHUNKS // GROUP
XP = 1024            # x DMA piece width
NPIECE = N // XP
PV_DELAY = 3         # groups between exp and its PV/ones matmuls
OUTPROJ_DELAY = 3    # further groups before a tile's output projection

NCORES = 8


def _kernel_body(tc, x_d, xb_d, wi_d, wo_d, out_d):
    nc = tc.nc
    with ExitStack() as es:
        res = es.enter_context(tc.tile_pool(name="res", bufs=1))

        ident = res.tile([P, P], BF16, tag="ident")
        make_identity(nc, ident[:])
        ones = res.tile([P, 2, 16], FP8V, tag="ones")  # [:, :, 0:1]; 16B step
        nc.gpsimd.memset(ones[:], 1.0)
        ebias = res.tile([P, 1], FP32, tag="ebias")
        nc.gpsimd.memset(ebias[:], EXP_BIAS)

        wi_sb = res.tile([P, CCH, 3 * CR], BF16, tag="wi")
        for k in range(CCH):
            nc.gpsimd.dma_start(wi_sb[:, k, :], wi_d[k * P:(k + 1) * P, :])
        wo_sb = res.tile([P, C], BF16, tag="wo")
        nc.gpsimd.dma_start(wo_sb[:], wo_d[:, :])

        # bf16 x pieces (contiguous in DRAM), ordered piece-major so the first
        # projection tiles' inputs arrive first
        xb = [res.tile([P, N], BF16, tag=f"xb{k}", name=f"xb{k}") for k in range(CCH)]
        for pi in range(NPIECE):
            for k in range(CCH):
                nc.sync.dma_start(xb[k][:, pi * XP:(pi + 1) * XP], xb_d[k, pi])
        # fp32 x (residual, needed late): same queue, behind the bf16 pieces
        x_sb = []
        for k in range(CCH):
            t = res.tile([P, N], FP32, tag=f"x{k}")
            nc.sync.dma_start(t[:], x_d[k * P:(k + 1) * P, :])
            x_sb.append(t)

        q_sb = res.tile([P, N], BF16, tag="q")
        k_sb = res.tile([P, N], BF16, tag="k")
        v_sb = res.tile([P, N], BF16, tag="v")
        qkv = [q_sb, k_sb, v_sb]
        vt_sb = res.tile([P, M_CHUNKS, P], FP8V, tag="vt")

        stpool = es.enter_context(tc.tile_pool(name="st", bufs=2, space="PSUM"))
        ypool = es.enter_context(tc.tile_pool(name="yps", bufs=1, space="PSUM"))
        spool = es.enter_context(tc.tile_pool(name="sps", bufs=1, space="PSUM"))
        # shared pool: projection psum early, outproj z psum later (disjoint
        # lifetimes keep total PSUM at 8 banks)
        zpool = es.enter_context(tc.tile_pool(name="zz", bufs=2, space="PSUM"))
        ppool = es.enter_context(tc.tile_pool(name="pexp", bufs=5))
        scpool = es.enter_context(tc.tile_pool(name="sc", bufs=2))
        rbpool = es.enter_context(tc.tile_pool(name="rb", bufs=2))
        ynpool = es.enter_context(tc.tile_pool(name="yn", bufs=2))
        opool = es.enter_context(tc.tile_pool(name="ob", bufs=3))

        def emit_proj_tile(o, ti):
            ps = zpool.tile([P, NT], FP32, tag="z", name=f"mmps_{o}_{ti}")
            for k in range(CCH):
                nc.tensor.matmul(
                    ps[:],
                    wi_sb[:, k, o * CR:(o + 1) * CR],
                    xb[k][:, ti * NT:(ti + 1) * NT],
                    start=(k == 0),
                    stop=(k == CCH - 1),
                )
            nc.vector.tensor_copy(qkv[o][:, ti * NT:(ti + 1) * NT], ps[:])

        def emit_vt(j):
            vps = zpool.tile([P, P], BF16, tag="z", name=f"vtps_{j}")
            nc.tensor.transpose(vps[:], v_sb[:, j * P:(j + 1) * P], ident[:])
            nc.vector.tensor_copy(vt_sb[:, j, :], vps[:])

        # deadline-ordered projection work drip-fed into the attention loop
        # (deadline = attention iteration index before which it must be emitted)
        work = []
        for i in range(1, N_TILES):
            work.append((2 * i - 1, "K", i))      # K tile i used by group 2i
        for i in range(2, N_TILES):
            work.append((2 * i - 1, "V", i))      # V tile i feeds vt 4i..4i+3
        for j in range(4, M_CHUNKS):
            work.append((j // 2 + 1, "vt", j))    # vt pair used at flush j//2+2
        for t in range(1, N_TILES):
            work.append((16 * t - 2, "Q", t))
        work.sort(key=lambda w: w[0])

        state = {}   # ti -> (y_ps, s_ps)
        pending = []  # exp'd groups waiting for PV/ones
        tail_queue = []  # (due_idx, ti, yn)
        cur_idx = [0]

        def emit_normalize(ti):
            y_ps, s_ps = state.pop(ti)
            yf = rbpool.tile([P, NT], FP32, tag="yf")
            nc.vector.tensor_copy(yf[:], y_ps[:])  # releases y bank
            sc = scpool.tile([1, NT], FP32, tag="sc")
            nc.vector.tensor_copy(sc[:], s_ps[0:1, :])  # releases s bank
            rb = rbpool.tile([P, NT], FP32, tag="rb")
            nc.gpsimd.partition_broadcast(rb[:], sc[:])
            nc.vector.reciprocal_approx_fast(rb[:], rb[:])
            yn = ynpool.tile([P, NT], BF16, tag="yn")
            nc.vector.tensor_mul(yn[:], yf[:], rb[:])
            return yn

        def emit_outproj(ti, yn, o):
            """one output-channel chunk of tile ti (spread across iterations)"""
            nsl = slice(ti * NT, (ti + 1) * NT)
            z_ps = zpool.tile([P, NT], FP32, tag="z")
            nc.tensor.matmul(
                z_ps[:],
                wo_sb[:, o * P:(o + 1) * P],
                yn[:],
                start=True,
                stop=True,
            )
            o_sb = opool.tile([P, NT], FP32, tag="ob")
            nc.vector.tensor_add(o_sb[:], z_ps[:], x_sb[o][:, nsl])
            nc.sync.dma_start(out_d[o * P:(o + 1) * P, nsl], o_sb[:])

        def flush_one():
            ti, j0, pexp = pending.pop(0)
            y_ps, s_ps = state[ti]
            nc.tensor.matmul(
                y_ps[:],
                vt_sb[:, j0:j0 + 2, :],
                pexp[:],
                start=(j0 == 0),
                stop=(j0 + 2 == M_CHUNKS),
                perf_mode=mybir.MatmulPerfMode.DoubleRow,
            )
            nc.tensor.matmul(
                s_ps[:],
                ones[:, :, 0:1],
                pexp[:],
                start=(j0 == 0),
                stop=(j0 + 2 == M_CHUNKS),
                perf_mode=mybir.MatmulPerfMode.DoubleRow,
            )
            if j0 + GROUP == M_CHUNKS:
                yn = emit_normalize(ti)
                for o in range(CCH):
                    tail_queue.append((cur_idx[0] + OUTPROJ_DELAY + o, ti, yn, o))

        # prologue: V tiles 0-1, Q tile 0, K tile 0, first vt chunks
        emit_proj_tile(2, 0)
        emit_proj_tile(2, 1)
        emit_proj_tile(0, 0)
        emit_proj_tile(1, 0)
        for j in range(4):
            emit_vt(j)

        for ti in range(N_TILES):
            nsl = slice(ti * NT, (ti + 1) * NT)
            state[ti] = (
                ypool.tile([P, NT], FP32, tag="y", name=f"y_{ti}"),
                spool.tile([1, NT], FP32, tag="s", name=f"s_{ti}"),
            )
            for g in range(N_GROUPS):
                idx = cur_idx[0]
                while work and work[0][0] <= idx:
                    _, kind, arg = work.pop(0)
                    if kind == "K":
                        emit_proj_tile(1, arg)
                    elif kind == "V":
                        emit_proj_tile(2, arg)
                    elif kind == "Q":
                        emit_proj_tile(0, arg)
                    else:
                        emit_vt(arg)
                endgame = ti == N_TILES - 1 and g >= N_GROUPS - 3
                while tail_queue and tail_queue[0][0] <= (
                    idx + OUTPROJ_DELAY if endgame else idx
                ):
                    _, tti, yn, o = tail_queue.pop(0)
                    emit_outproj(tti, yn, o)
                j0 = g * GROUP
                st = stpool.tile([P, GROUP, NT], FP32, tag="st")
                for i in range(GROUP):
                    nc.tensor.matmul(
                        st[:, i, :],
                        k_sb[:, (j0 + i) * P:(j0 + i + 1) * P],
                        q_sb[:, nsl],
                        start=True,
                        stop=True,
                    )
                pexp = ppool.tile([P, GROUP, NT], FP8W, tag="p")
                nc.scalar.activation(
                    pexp[:].rearrange("p a b -> p (a b)"),
                    st[:].rearrange("p a b -> p (a b)"),
                    mybir.ActivationFunctionType.Exp,
                    bias=ebias[:],
                )
                pending.append((ti, j0, pexp))
                limit = 0 if endgame else PV_DELAY
                while len(pending) > limit:
                    flush_one()
                cur_idx[0] += 1
        while pending:
            flush_one()
        while tail_queue:
            _, tti, yn, o = tail_queue.pop(0)
            emit_outproj(tti, yn, o)


def build_program():
    nc = bacc.Bacc("TRN2", target_bir_lowering=False, debug=False)
    x_d = nc.dram_tensor("x", [C, N], FP32, kind="ExternalInput").ap()
    xb_d = nc.dram_tensor("xbf", [CCH, NPIECE, P, XP], BF16, kind="ExternalInput").ap()
    wi_d = nc.dram_tensor("w_inT", [C, 3 * CR], BF16, kind="ExternalInput").ap()
    wo_d = nc.dram_tensor("w_outT", [CR, C], BF16, kind="ExternalInput").ap()
    out_d = nc.dram_tensor("out", [C, N], FP32, kind="ExternalOutput").ap()
    with tile.TileContext(nc) as tc:
        _kernel_body(tc, x_d, xb_d, wi_d, wo_d, out_d)
    nc.compile()
    return nc


_CACHED_NC = None


def _get_nc():
    global _CACHED_NC
    if _CACHED_NC is None:
        _CACHED_NC = build_program()
    return _CACHED_NC


def tile_xbf(xf_b):
    """[C, N] -> [CCH, NPIECE, P, XP] bf16, each piece contiguous."""
    t = xf_b.reshape(CCH, P, NPIECE, XP).transpose(0, 2, 1, 3)
    return np.ascontiguousarray(t).astype(ml_dtypes.bfloat16)


def make_in_maps(x, w_in, w_out):
    x = np.asarray(x)
    w_in = np.asarray(w_in, dtype=np.float32)
    w_out = np.asarray(w_out, dtype=np.float32)
    xf = np.ascontiguousarray(x.reshape(B, C, N), dtype=np.float32)
    wiT = np.ascontiguousarray(w_in.T).astype(ml_dtypes.bfloat16)
    woT = np.ascontiguousarray(w_out.T).astype(ml_dtypes.bfloat16)
    return [
        {
            "x": np.ascontiguousarray(xf[b]),
            "xbf": tile_xbf(xf[b]),
            "w_inT": wiT,
            "w_outT": woT,
        }
        for b in range(B)
    ]


def kernel(x, w_in, w_out):
    nc = _get_nc()
    in_maps = make_in_maps(x, w_in, w_out)
    trace = bool(int(os.environ.get("KERNEL_TRACE", "0")))
    res = run_bass_kernel_spmd(nc, in_maps, list(range(NCORES)), trace=trace)
    if trace and res.exec_time_ns is not None:
        print(f"HW exec time: {res.exec_time_ns} ns")
        if res.instructions_and_trace is not None:
            print(f"trace: {res.instructions_and_trace[1]}")
    out = np.stack([res.results[b]["out"] for b in range(B)], axis=0)
    return out.reshape(B, C, HH, WW).astype(np.float32)


# revision 40
# speedup vs baseline: 1.0086x; 1.0086x over previous
"""NonLocalBlock (spatial self-attention) Trainium2 Bass kernel.

Data-parallel over batch: B=8 -> one batch element per NeuronCore.

Per-core computation (C=512, CR=128, N=4096 = 64*64 tokens), all on one core:
  proj = w_in @ x          -> [384, N]; Q=proj[0:128], K=proj[128:256], V=proj[256:384]
  S^T[m,n] = sum_c K[c,m] Q[c,n]    (tiles: m on partitions, n on free axis)
  P = exp(S^T - 8)         (no max subtraction; logits are ~N(0, 2.3), max ~16;
                            the -8 keeps fp8e5m2 away from overflow-to-inf and
                            cancels exactly in y/s)
  s[n] = sum_m P[m,n]      (fp8 DoubleRow ones-matmuls, PSUM-accumulated)
  yT[c,n] = sum_m V^T[m,c] P[m,n]   (fp8 DoubleRow, PSUM-accumulated)
  out = x + w_out @ (yT / s)

Engine plan: QK runs in bf16 (logit precision), the exp output is fp8e5m2 so
PV and the softmax sums contract two m-chunks per DoubleRow matmul. The whole
kernel is one software-pipelined loop: the input projection and V^T transpose
work items carry deadlines and are drip-fed into the attention loop, so the
PE works through them while ScalarE streams exps; PV trails its exp by
PV_DELAY groups; each n-tile's normalize + output projection trail further.
The host supplies x twice: bf16 in a DMA-friendly tiled layout (feeds the
projection) and fp32 row-major (residual, needed much later).
"""

import os
from contextlib import ExitStack

import numpy as np
import ml_dtypes

import concourse.tile as tile
from concourse import bacc, mybir
from concourse.bass_utils import run_bass_kernel_spmd
from concourse.masks import make_identity

FP32 = mybir.dt.float32
BF16 = mybir.dt.bfloat16
FP8W = mybir.dt.float8e5  # e5m2: exp weights (wide dynamic range)
FP8V = mybir.dt.float8e4  # e4m3: V^T values
EXP_BIAS = -8.0

B, C, HH, WW = 8, 512, 64, 64
N = HH * WW          # 4096 spatial tokens
CR = 128             # reduced channels (= partition count, exact fit)
P = 128              # SBUF partitions
CCH = C // P         # 4 chunks of input channels
NT = 512             # n-tile width (one PSUM bank of fp32)
N_TILES = N // NT    # 8
M_CHUNKS = N // P    # 32 chunks of the m (key/value token) axis
GROUP = 2            # m-chunks per exp batch (one fp8 DoubleRow pair)
N_GROUPS = M_CHUNKS // GROUP
XP = 1024            # x DMA piece width
NPIECE = N // XP
PV_DELAY = 3         # groups between exp and its PV/ones matmuls
OUTPROJ_DELAY = 3    # further groups before a tile's output projection

NCORES = 8


def _kernel_body(tc, x_d, xb_d, wi_d, wo_d, out_d):
    nc = tc.nc
    with ExitStack() as es:
        res = es.enter_context(tc.tile_pool(name="res", bufs=1))

        ident = res.tile([P, P], BF16, tag="ident")
        make_identity(nc, ident[:])
        ones = res.tile([P, 2, 16], FP8V, tag="ones")  # [:, :, 0:1]; 16B step
        nc.gpsimd.memset(ones[:], 1.0)
        ebias = res.tile([P, 1], FP32, tag="ebias")
        nc.gpsimd.memset(ebias[:], EXP_BIAS)
        # dummy activation: pulls the ~2.7us exp table load into the DMA wait
        warm = res.tile([P, 1], FP32, tag="warm")
        nc.scalar.activation(warm[:], ebias[:], mybir.ActivationFunctionType.Exp)

        wi_sb = res.tile([P, CCH, 3 * CR], BF16, tag="wi")
        for k in range(CCH):
            nc.gpsimd.dma_start(wi_sb[:, k, :], wi_d[k * P:(k + 1) * P, :])
        wo_sb = res.tile([P, C], BF16, tag="wo")
        nc.gpsimd.dma_start(wo_sb[:], wo_d[:, :])

        # bf16 x pieces (contiguous in DRAM), ordered piece-major so the first
        # projection tiles' inputs arrive first
        xb = [res.tile([P, N], BF16, tag=f"xb{k}", name=f"xb{k}") for k in range(CCH)]
        for pi in range(NPIECE):
            for k in range(CCH):
                nc.sync.dma_start(xb[k][:, pi * XP:(pi + 1) * XP], xb_d[k, pi])
        # fp32 x (residual, needed late): same queue, behind the bf16 pieces
        x_sb = []
        for k in range(CCH):
            t = res.tile([P, N], FP32, tag=f"x{k}")
            nc.sync.dma_start(t[:], x_d[k * P:(k + 1) * P, :])
            x_sb.append(t)

        q_sb = res.tile([P, N], BF16, tag="q")
        k_sb = res.tile([P, N], BF16, tag="k")
        v_sb = res.tile([P, N], BF16, tag="v")
        qkv = [q_sb, k_sb, v_sb]
        vt_sb = res.tile([P, M_CHUNKS, P], FP8V, tag="vt")

        stpool = es.enter_context(tc.tile_pool(name="st", bufs=2, space="PSUM"))
        ypool = es.enter_context(tc.tile_pool(name="yps", bufs=1, space="PSUM"))
        spool = es.enter_context(tc.tile_pool(name="sps", bufs=1, space="PSUM"))
        # shared pool: projection psum early, outproj z psum later (disjoint
        # lifetimes keep total PSUM at 8 banks)
        zpool = es.enter_context(tc.tile_pool(name="zz", bufs=2, space="PSUM"))
        ppool = es.enter_context(tc.tile_pool(name="pexp", bufs=5))
        scpool = es.enter_context(tc.tile_pool(name="sc", bufs=2))
        rbpool = es.enter_context(tc.tile_pool(name="rb", bufs=2))
        ynpool = es.enter_context(tc.tile_pool(name="yn", bufs=2))
        opool = es.enter_context(tc.tile_pool(name="ob", bufs=3))

        def emit_proj_tile(o, ti):
            ps = zpool.tile([P, NT], FP32, tag="z", name=f"mmps_{o}_{ti}")
            for k in range(CCH):
                nc.tensor.matmul(
                    ps[:],
                    wi_sb[:, k, o * CR:(o + 1) * CR],
                    xb[k][:, ti * NT:(ti + 1) * NT],
                    start=(k == 0),
                    stop=(k == CCH - 1),
                )
            nc.vector.tensor_copy(qkv[o][:, ti * NT:(ti + 1) * NT], ps[:])

        def emit_vt(j):
            vps = zpool.tile([P, P], BF16, tag="z", name=f"vtps_{j}")
            nc.tensor.transpose(vps[:], v_sb[:, j * P:(j + 1) * P], ident[:])
            nc.vector.tensor_copy(vt_sb[:, j, :], vps[:])

        # deadline-ordered projection work drip-fed into the attention loop
        # (deadline = attention iteration index before which it must be emitted)
        work = [(0, "V", 0), (0, "V", 1)]
        for j in range(4):
            work.append((1 + j // 2, "vt", j))
        for i in range(1, N_TILES):
            work.append((2 * i - 1, "K", i))      # K tile i used by group 2i
        for i in range(2, N_TILES):
            work.append((2 * i - 1, "V", i))      # V tile i feeds vt 4i..4i+3
        for j in range(4, M_CHUNKS):
            work.append((j // 2 + 1, "vt", j))    # vt pair used at flush j//2+PV_DELAY
        for t in range(1, N_TILES):
            work.append((16 * t - 2, "Q", t))
        work.sort(key=lambda w: w[0])

        state = {}   # ti -> (y_ps, s_ps)
        pending = []  # exp'd groups waiting for PV/ones
        tail_queue = []  # (due_idx, ti, yn)
        cur_idx = [0]

        def emit_normalize(ti):
            y_ps, s_ps = state.pop(ti)
            last = ti == N_TILES - 1
            if not last:
                yf = rbpool.tile([P, NT], FP32, tag="yf")
                nc.vector.tensor_copy(yf[:], y_ps[:])  # releases y bank early
            sc = scpool.tile([1, NT], FP32, tag="sc")
            nc.vector.tensor_copy(sc[:], s_ps[0:1, :])  # releases s bank
            rb = rbpool.tile([P, NT], FP32, tag="rb")
            nc.gpsimd.partition_broadcast(rb[:], sc[:])
            nc.vector.reciprocal_approx_fast(rb[:], rb[:])
            yn = ynpool.tile([P, NT], BF16, tag="yn")
            # last tile: no reuse pressure on the y bank, skip the copy hop
            nc.vector.tensor_mul(yn[:], y_ps[:] if last else yf[:], rb[:])
            return yn

        def emit_outproj(ti, yn, o):
            """one output-channel chunk of tile ti (spread across iterations)"""
            nsl = slice(ti * NT, (ti + 1) * NT)
            z_ps = zpool.tile([P, NT], FP32, tag="z")
            nc.tensor.matmul(
                z_ps[:],
                wo_sb[:, o * P:(o + 1) * P],
                yn[:],
                start=True,
                stop=True,
            )
            o_sb = opool.tile([P, NT], FP32, tag="ob")
            nc.vector.tensor_add(o_sb[:], z_ps[:], x_sb[o][:, nsl])
            nc.sync.dma_start(out_d[o * P:(o + 1) * P, nsl], o_sb[:])

        def flush_one():
            ti, j0, pexp = pending.pop(0)
            y_ps, s_ps = state[ti]
            nc.tensor.matmul(
                y_ps[:],
                vt_sb[:, j0:j0 + 2, :],
                pexp[:],
                start=(j0 == 0),
                stop=(j0 + 2 == M_CHUNKS),
                perf_mode=mybir.MatmulPerfMode.DoubleRow,
            )
            nc.tensor.matmul(
                s_ps[:],
                ones[:, :, 0:1],
                pexp[:],
                start=(j0 == 0),
                stop=(j0 + 2 == M_CHUNKS),
                perf_mode=mybir.MatmulPerfMode.DoubleRow,
            )
            if j0 + GROUP == M_CHUNKS:
                yn = emit_normalize(ti)
                for o in range(CCH):
                    tail_queue.append((cur_idx[0] + OUTPROJ_DELAY + o, ti, yn, o))

        # prologue: only what the first QK needs (V/vt arrive via the work queue)
        emit_proj_tile(0, 0)
        emit_proj_tile(1, 0)

        for ti in range(N_TILES):
            nsl = slice(ti * NT, (ti + 1) * NT)
            state[ti] = (
                ypool.tile([P, NT], FP32, tag="y", name=f"y_{ti}"),
                spool.tile([1, NT], FP32, tag="s", name=f"s_{ti}"),
            )
            for g in range(N_GROUPS):
                idx = cur_idx[0]
                while work and work[0][0] <= idx:
                    _, kind, arg = work.pop(0)
                    if kind == "K":
                        emit_proj_tile(1, arg)
                    elif kind == "V":
                        emit_proj_tile(2, arg)
                    elif kind == "Q":
                        emit_proj_tile(0, arg)
                    else:
                        emit_vt(arg)
                endgame = ti == N_TILES - 1 and g >= N_GROUPS - 3
                while tail_queue and tail_queue[0][0] <= (
                    idx + OUTPROJ_DELAY if endgame else idx
                ):
                    _, tti, yn, o = tail_queue.pop(0)
                    emit_outproj(tti, yn, o)
                j0 = g * GROUP
                st = stpool.tile([P, GROUP, NT], FP32, tag="st")
                for i in range(GROUP):
                    nc.tensor.matmul(
                        st[:, i, :],
                        k_sb[:, (j0 + i) * P:(j0 + i + 1) * P],
                        q_sb[:, nsl],
                        start=True,
                        stop=True,
                    )
                pexp = ppool.tile([P, GROUP, NT], FP8W, tag="p")
                nc.scalar.activation(
                    pexp[:].rearrange("p a b -> p (a b)"),
                    st[:].rearrange("p a b -> p (a b)"),
                    mybir.ActivationFunctionType.Exp,
                    bias=ebias[:],
                )
                pending.append((ti, j0, pexp))
                limit = 0 if endgame else PV_DELAY
                while len(pending) > limit:
                    flush_one()
                cur_idx[0] += 1
        while pending:
            flush_one()
        while tail_queue:
            _, tti, yn, o = tail_queue.pop(0)
            emit_outproj(tti, yn, o)


def build_program():
    nc = bacc.Bacc("TRN2", target_bir_lowering=False, debug=False)
    x_d = nc.dram_tensor("x", [C, N], FP32, kind="ExternalInput").ap()
    xb_d = nc.dram_tensor("xbf", [CCH, NPIECE, P, XP], BF16, kind="ExternalInput").ap()
    wi_d = nc.dram_tensor("w_inT", [C, 3 * CR], BF16, kind="ExternalInput").ap()
    wo_d = nc.dram_tensor("w_outT", [CR, C], BF16, kind="ExternalInput").ap()
    out_d = nc.dram_tensor("out", [C, N], FP32, kind="ExternalOutput").ap()
    with tile.TileContext(nc) as tc:
        _kernel_body(tc, x_d, xb_d, wi_d, wo_d, out_d)
    nc.compile()
    return nc


_CACHED_NC = None


def _get_nc():
    global _CACHED_NC
    if _CACHED_NC is None:
        _CACHED_NC = build_program()
    return _CACHED_NC


def tile_xbf(xf_b):
    """[C, N] -> [CCH, NPIECE, P, XP] bf16, each piece contiguous."""
    t = xf_b.reshape(CCH, P, NPIECE, XP).transpose(0, 2, 1, 3)
    return np.ascontiguousarray(t).astype(ml_dtypes.bfloat16)


def make_in_maps(x, w_in, w_out):
    x = np.asarray(x)
    w_in = np.asarray(w_in, dtype=np.float32)
    w_out = np.asarray(w_out, dtype=np.float32)
    xf = np.ascontiguousarray(x.reshape(B, C, N), dtype=np.float32)
    wiT = np.ascontiguousarray(w_in.T).astype(ml_dtypes.bfloat16)
    woT = np.ascontiguousarray(w_out.T).astype(ml_dtypes.bfloat16)
    return [
        {
            "x": np.ascontiguousarray(xf[b]),
            "xbf": tile_xbf(xf[b]),
            "w_inT": wiT,
            "w_outT": woT,
        }
        for b in range(B)
    ]


def kernel(x, w_in, w_out):
    nc = _get_nc()
    in_maps = make_in_maps(x, w_in, w_out)
    trace = bool(int(os.environ.get("KERNEL_TRACE", "0")))
    res = run_bass_kernel_spmd(nc, in_maps, list(range(NCORES)), trace=trace)
    if trace and res.exec_time_ns is not None:
        print(f"HW exec time: {res.exec_time_ns} ns")
        if res.instructions_and_trace is not None:
            print(f"trace: {res.instructions_and_trace[1]}")
    out = np.stack([res.results[b]["out"] for b in range(B)], axis=0)
    return out.reshape(B, C, HH, WW).astype(np.float32)
